# revision 5
# baseline (speedup 1.0000x reference)
"""Trainium2 Bass kernel for EqualizedModConv2d (StyleGAN2 modulated conv).

Math (per sample b):
  s[i]  = (y @ dense_w.T) * LATENT**-0.5 + dense_b                 # style
  ww    = weight * W_MUL * s[i]                                    # modulate
  d[o]  = rsqrt(sum_{i,k}(ww^2) + 1e-8)                           # demodulate
  out   = conv2d(x, ww * d, VALID)

Kernel decomposition (demod as pre/post scaling — weights stay sample-invariant):
  out[b,o,p] = D[b,o] * sum_{i,k} w[o,i,k] * (s[b,i] * x[b,i,p+k])
  D[b,o]     = 1/sqrt(T[b,o] + 1e-8/W_MUL^2),  T = (s*s) @ wsq.T,
  wsq[o,i]   = sum_k w[o,i,k]^2
(identical to the reference formula: mul*rsqrt(mul^2*T + eps) == rsqrt(T + eps/mul^2))

Distribution: data-parallel over batch, 2 samples per core on 8 cores.
Conv runs as 36 accumulating PE matmuls (9 taps x 4 cin-chunks) per
(sample, cout-tile, row-tile) in bf16 with fp32 PSUM accumulation.
Weights ship pre-transposed in bf16 (the dtype the PE consumes); demod is
computed from the same bf16 weights the conv uses. x ships fp32 and is
style-scaled + converted on device.
"""

import sys
import types

import numpy as np

B, CIN, COUT, LATENT = 16, 512, 512, 512
H = W = 64
KH = KW = 3
OH = OW = 62
N_CORES = 8
BL = B // N_CORES  # samples per core
IC = CIN // 128    # cin chunks
OC = COUT // 128   # cout tiles
MUL_DENSE = float(LATENT ** -0.5)
# rsqrt(W_MUL^2 * T + 1e-8) * W_MUL == rsqrt(T + 1e-8 / W_MUL^2)
EPS2 = float(1e-8 * (CIN * KH * KW))
ROW_STARTS = [0, 8, 16, 24, 32, 40, 48, 56]
ROW_COUNTS = [8, 8, 8, 8, 8, 8, 8, 6]

_cache = {}


def _ensure_ntff_hook():
    """The agent image's antenv lacks axon_hooks, so boot silently skipped NTFF
    hook registration; recreate the module + hook so trace=True works."""
    if "antenv.axon_hooks" in sys.modules:
        return
    try:
        import antenv
        from trn_agent_boot.trn_boot import _ntff_profile_via_ctypes
    except ImportError:
        return
    mod = types.ModuleType("antenv.axon_hooks")
    mod._hook = None

    def _set(h):
        mod._hook = h

    def _get():
        return mod._hook

    mod.set_axon_ntff_profile_hook = _set
    mod.get_axon_ntff_profile_hook = _get
    sys.modules["antenv.axon_hooks"] = mod
    antenv.axon_hooks = mod
    try:
        _set(_ntff_profile_via_ctypes("/opt/axon/libaxon_pjrt.so"))
    except OSError:
        pass


def build():
    """Emit + compile the per-core Tile program. Returns the Bass object."""
    import concourse.bass as bass
    import concourse.bacc as bacc
    import concourse.tile as tile
    from concourse import mybir

    f32 = mybir.dt.float32
    bf16 = mybir.dt.bfloat16
    AF = mybir.ActivationFunctionType
    PSUM = bass.MemorySpace.PSUM

    nc = bacc.Bacc("TRN2", target_bir_lowering=False, debug=False)

    x_d = nc.dram_tensor("x", [BL, CIN, H, W], f32, kind="ExternalInput")
    wtb_d = nc.dram_tensor("wtb", [CIN, 9, COUT], bf16, kind="ExternalInput")  # [i,k,o]
    dwt_d = nc.dram_tensor("dwt", [LATENT, CIN], f32, kind="ExternalInput")    # [l,c]
    yt_d = nc.dram_tensor("yt", [LATENT, BL], f32, kind="ExternalInput")       # [l,b]
    db_d = nc.dram_tensor("db", [CIN, 1], f32, kind="ExternalInput")
    out_d = nc.dram_tensor("out", [BL, COUT, OH, OW], f32, kind="ExternalOutput")

    with tile.TileContext(nc) as tc:
        with (
            tc.tile_pool(name="persist", bufs=1) as persist,
            tc.tile_pool(name="sq", bufs=1) as sqpool,
            tc.tile_pool(name="xstage", bufs=3) as xstage,
            tc.tile_pool(name="osb", bufs=6) as osb,
            tc.tile_pool(name="small", bufs=1) as small,
            tc.tile_pool(name="psc", bufs=7, space=PSUM) as psc,
            tc.tile_pool(name="pss", bufs=1, space=PSUM) as pss,
        ):
            # ---- persistent SBUF tensors ----
            wbf = persist.tile([128, IC, 9, COUT], bf16)     # conv weights [i, (ic,k,o)]
            xs = persist.tile([128, BL, IC, H, W], bf16)     # style-scaled inputs
            wsq = persist.tile([128, IC, COUT], f32)         # sum_k w^2, [i,(ic,o)]

            # ---- style: s = (y @ dense_w.T) * mul + b ----
            dwt_sb = small.tile([128, 4, CIN], f32)
            nc.sync.dma_start(dwt_sb[:, :, :], dwt_d.ap().rearrange("(l p) c -> p l c", p=128))
            yt_sb = small.tile([128, 4, BL], f32)
            nc.sync.dma_start(yt_sb[:, :, :], yt_d.ap().rearrange("(l p) b -> p l b", p=128))
            db_sb = small.tile([128, 4, 1], f32)
            nc.sync.dma_start(db_sb[:, :, :], db_d.ap().rearrange("(c p) u -> p c u", p=128))

            s_sb = small.tile([128, IC, BL], f32)
            ssq = small.tile([128, IC, BL], f32)
            for ct in range(IC):
                ps = pss.tile([128, BL], f32)
                for lc in range(4):
                    nc.tensor.matmul(
                        ps[:, :],
                        dwt_sb[:, lc, ct * 128:(ct + 1) * 128],
                        yt_sb[:, lc, :],
                        start=(lc == 0),
                        stop=(lc == 3),
                    )
                nc.scalar.activation(
                    s_sb[:, ct, :], ps[:, :], AF.Identity,
                    bias=db_sb[:, ct, :], scale=MUL_DENSE,
                )
                nc.vector.tensor_mul(ssq[:, ct, :], s_sb[:, ct, :], s_sb[:, ct, :])

            # ---- conv weights: direct bf16 DMA (critical path) ----
            for ic in range(IC):
                nc.sync.dma_start(wbf[:, ic, :, :], wtb_d[ic * 128:(ic + 1) * 128, :, :])

            # ---- x sample 0: fp32 load halves, scale by s + convert on ACT ----
            for hh in range(2):
                for ic in range(IC):
                    h0 = hh * 32
                    xr = xstage.tile([128, 32, W], f32, tag="xr")
                    nc.sync.dma_start(xr[:, :, :], x_d[0, ic * 128:(ic + 1) * 128, h0:h0 + 32, :])
                    nc.scalar.activation(
                        xs[:, 0, ic, h0:h0 + 32, :], xr[:, :, :], AF.Copy,
                        scale=s_sb[:, ic, 0:1],
                    )

            # ---- x sample 1: loads now (DMA order), scale+convert on DVE ----
            def emit_x1():
                for hh in range(2):
                    for ic in range(IC):
                        h0 = hh * 32
                        xr = xstage.tile([128, 32, W], f32, tag="xr")
                        nc.sync.dma_start(xr[:, :, :], x_d[1, ic * 128:(ic + 1) * 128, h0:h0 + 32, :])
                        nc.vector.tensor_scalar_mul(
                            xs[:, 1, ic, h0:h0 + 32, :], xr[:, :, :], s_sb[:, ic, 1:2],
                        )

            # ---- wsq[i,o] = sum_k wbf[i,k,o]^2  (from the same bf16 the conv uses) ----
            for ic in range(IC):
                sq = sqpool.tile([128, 9, COUT], f32)
                nc.vector.tensor_mul(sq[:, :, :], wbf[:, ic, :, :], wbf[:, ic, :, :])
                nc.vector.reduce_sum(
                    wsq[:, ic, :],
                    sq[:, :, :].rearrange("p k o -> p o k"),
                    axis=mybir.AxisListType.X,
                )

            # ---- demod: T = wsq.T @ ssq ; D = 1/sqrt(T + eps') ----
            sqrt_t = small.tile([128, OC, BL], f32)
            d_sb = small.tile([128, OC, BL], f32)
            eps_sb = small.tile([128, 1], f32)
            nc.vector.memset(eps_sb[:, :], EPS2)
            for ot in range(OC):
                ps = pss.tile([128, BL], f32)
                for ic in range(IC):
                    nc.tensor.matmul(
                        ps[:, :],
                        wsq[:, ic, ot * 128:(ot + 1) * 128],
                        ssq[:, ic, :],
                        start=(ic == 0),
                        stop=(ic == 3),
                    )
                nc.scalar.activation(
                    sqrt_t[:, ot, :], ps[:, :], AF.Sqrt,
                    bias=eps_sb[:, :], scale=1.0,
                )
                nc.vector.reciprocal(d_sb[:, ot, :], sqrt_t[:, ot, :])

            emit_x1()

            # ---- main conv: 36 accumulating matmuls per (b, ot, row-tile) ----
            for b in range(BL):
                for ot in range(OC):
                    o0 = ot * 128
                    for r0, rows in zip(ROW_STARTS, ROW_COUNTS):
                        ps = psc.tile([128, 8, OW], f32, tag="convps")
                        first = True
                        for ic in range(IC):
                            for kh in range(KH):
                                for kw in range(KW):
                                    last = ic == IC - 1 and kh == KH - 1 and kw == KW - 1
                                    nc.tensor.matmul(
                                        ps[:, :rows, :],
                                        wbf[:, ic, kh * 3 + kw, o0:o0 + 128],
                                        xs[:, b, ic, r0 + kh:r0 + kh + rows, kw:kw + OW],
                                        start=first,
                                        stop=last,
                                    )
                                    first = False
                        ob = osb.tile([128, 8, OW], f32, tag="outsb")
                        nc.scalar.activation(
                            ob[:, :rows, :], ps[:, :rows, :], AF.Copy,
                            scale=d_sb[:, ot, b:b + 1],
                        )
                        nc.sync.dma_start(
                            out_d[b, o0:o0 + 128, r0:r0 + rows, :], ob[:, :rows, :]
                        )

    nc.compile()
    return nc


def run(inputs, profile=False):
    """inputs: dict with full-size x/y/dense_w/dense_b/weight numpy arrays.
    Returns (out [16,512,62,62] float32, exec_time_ns or None)."""
    import ml_dtypes
    from concourse.bass_utils import run_bass_kernel_spmd

    if "nc" not in _cache:
        _cache["nc"] = build()
    nc = _cache["nc"]

    x = np.ascontiguousarray(np.asarray(inputs["x"], dtype=np.float32))
    y = np.ascontiguousarray(np.asarray(inputs["y"], dtype=np.float32))
    dense_w = np.asarray(inputs["dense_w"], dtype=np.float32)
    dense_b = np.asarray(inputs["dense_b"], dtype=np.float32)
    weight = np.asarray(inputs["weight"], dtype=np.float32)

    # weights pre-transposed to the [cin, tap, cout] layout the PE consumes,
    # in the bf16 the kernel computes with
    wtb = np.ascontiguousarray(
        weight.reshape(COUT, CIN, 9).transpose(1, 2, 0).astype(ml_dtypes.bfloat16))
    dwt = np.ascontiguousarray(dense_w.T)                          # [l, c]
    db = np.ascontiguousarray(dense_b.reshape(CIN, 1))

    in_maps = []
    for c in range(N_CORES):
        sl = slice(c * BL, (c + 1) * BL)
        in_maps.append({
            "x": x[sl],
            "wtb": wtb,
            "dwt": dwt,
            "yt": np.ascontiguousarray(y[sl].T),                   # [l, b]
            "db": db,
        })

    if profile:
        _ensure_ntff_hook()
    res = run_bass_kernel_spmd(
        nc, in_maps, core_ids=list(range(N_CORES)), trace=profile)
    out = np.concatenate([r["out"] for r in res.results], axis=0)
    return out, res.exec_time_ns


def kernel(**inputs) -> np.ndarray:
    out, _ = run(inputs)
    return out


# revision 6
# speedup vs baseline: 1.0069x; 1.0069x over previous
"""Trainium2 Bass kernel for EqualizedModConv2d (StyleGAN2 modulated conv).

Math (per sample b):
  s[i]  = (y @ dense_w.T) * LATENT**-0.5 + dense_b                 # style
  ww    = weight * W_MUL * s[i]                                    # modulate
  d[o]  = rsqrt(sum_{i,k}(ww^2) + 1e-8)                           # demodulate
  out   = conv2d(x, ww * d, VALID)

Kernel decomposition (demod as pre/post scaling — weights stay sample-invariant):
  out[b,o,p] = D[b,o] * sum_{i,k} w[o,i,k] * (s[b,i] * x[b,i,p+k])
  D[b,o]     = 1/sqrt(T[b,o] + 1e-8/W_MUL^2),  T = (s*s) @ wsq.T,
  wsq[o,i]   = sum_k w[o,i,k]^2
(identical to the reference formula: mul*rsqrt(mul^2*T + eps) == rsqrt(T + eps/mul^2))

Distribution: data-parallel over batch, 2 samples per core on 8 cores.
Conv runs as 36 accumulating PE matmuls (9 taps x 4 cin-chunks) per
(sample, cout-tile, row-tile) in bf16 with fp32 PSUM accumulation.
Weights ship pre-transposed in bf16 (the dtype the PE consumes); demod is
computed from the same bf16 weights the conv uses. x ships fp32 and is
style-scaled + converted on device.
"""

import sys
import types

import numpy as np

B, CIN, COUT, LATENT = 16, 512, 512, 512
H = W = 64
KH = KW = 3
OH = OW = 62
N_CORES = 8
BL = B // N_CORES  # samples per core
IC = CIN // 128    # cin chunks
OC = COUT // 128   # cout tiles
MUL_DENSE = float(LATENT ** -0.5)
# rsqrt(W_MUL^2 * T + 1e-8) * W_MUL == rsqrt(T + 1e-8 / W_MUL^2)
EPS2 = float(1e-8 * (CIN * KH * KW))
ROW_STARTS = [0, 8, 16, 24, 32, 40, 48, 56]
ROW_COUNTS = [8, 8, 8, 8, 8, 8, 8, 6]

_cache = {}


def _ensure_ntff_hook():
    """The agent image's antenv lacks axon_hooks, so boot silently skipped NTFF
    hook registration; recreate the module + hook so trace=True works."""
    if "antenv.axon_hooks" in sys.modules:
        return
    try:
        import antenv
        from trn_agent_boot.trn_boot import _ntff_profile_via_ctypes
    except ImportError:
        return
    mod = types.ModuleType("antenv.axon_hooks")
    mod._hook = None

    def _set(h):
        mod._hook = h

    def _get():
        return mod._hook

    mod.set_axon_ntff_profile_hook = _set
    mod.get_axon_ntff_profile_hook = _get
    sys.modules["antenv.axon_hooks"] = mod
    antenv.axon_hooks = mod
    try:
        _set(_ntff_profile_via_ctypes("/opt/axon/libaxon_pjrt.so"))
    except OSError:
        pass


def build():
    """Emit + compile the per-core Tile program. Returns the Bass object."""
    import concourse.bass as bass
    import concourse.bacc as bacc
    import concourse.tile as tile
    from concourse import mybir

    f32 = mybir.dt.float32
    bf16 = mybir.dt.bfloat16
    AF = mybir.ActivationFunctionType
    PSUM = bass.MemorySpace.PSUM

    nc = bacc.Bacc("TRN2", target_bir_lowering=False, debug=False)

    x_d = nc.dram_tensor("x", [BL, CIN, H, W], f32, kind="ExternalInput")
    wtb_d = nc.dram_tensor("wtb", [CIN, 9, COUT], bf16, kind="ExternalInput")  # [i,k,o]
    dwt_d = nc.dram_tensor("dwt", [LATENT, CIN], f32, kind="ExternalInput")    # [l,c]
    yt_d = nc.dram_tensor("yt", [LATENT, BL], f32, kind="ExternalInput")       # [l,b]
    db_d = nc.dram_tensor("db", [CIN, 1], f32, kind="ExternalInput")
    out_d = nc.dram_tensor("out", [BL, COUT, OH, OW], f32, kind="ExternalOutput")

    with tile.TileContext(nc) as tc:
        with (
            tc.tile_pool(name="persist", bufs=1) as persist,
            tc.tile_pool(name="sq", bufs=1) as sqpool,
            tc.tile_pool(name="xstage", bufs=4) as xstage,
            tc.tile_pool(name="osb", bufs=6) as osb,
            tc.tile_pool(name="small", bufs=1) as small,
            tc.tile_pool(name="psc", bufs=7, space=PSUM) as psc,
            tc.tile_pool(name="pss", bufs=1, space=PSUM) as pss,
        ):
            # ---- persistent SBUF tensors ----
            wbf = persist.tile([128, IC, 9, COUT], bf16)     # conv weights [i, (ic,k,o)]
            xs = persist.tile([128, BL, IC, H, W], bf16)     # style-scaled inputs
            wsq = persist.tile([128, IC, COUT], f32)         # sum_k w^2, [i,(ic,o)]

            # ---- small param DMAs (own queues, land fast) ----
            dwt_sb = small.tile([128, 4, CIN], f32)
            nc.sync.dma_start(dwt_sb[:, :, :], dwt_d.ap().rearrange("(l p) c -> p l c", p=128))
            yt_sb = small.tile([128, 4, BL], f32)
            nc.sync.dma_start(yt_sb[:, :, :], yt_d.ap().rearrange("(l p) b -> p l b", p=128))
            db_sb = small.tile([128, 4, 1], f32)
            nc.sync.dma_start(db_sb[:, :, :], db_d.ap().rearrange("(c p) u -> p c u", p=128))

            # ---- conv weights: bf16 DMA in 12 chunks (spread across queues) +
            #      wsq[i,o] = sum_k wbf[i,k,o]^2 on DVE (first in DVE stream) ----
            for ic in range(IC):
                i0 = ic * 128
                for g in range(3):
                    nc.sync.dma_start(
                        wbf[:, ic, g * 3:(g + 1) * 3, :],
                        wtb_d[i0:i0 + 128, g * 3:(g + 1) * 3, :],
                    )
                sq = sqpool.tile([128, 9, COUT], bf16)
                nc.vector.tensor_mul(sq[:, :, :], wbf[:, ic, :, :], wbf[:, ic, :, :])
                nc.vector.reduce_sum(
                    wsq[:, ic, :],
                    sq[:, :, :].rearrange("p k o -> p o k"),
                    axis=mybir.AxisListType.X,
                )

            # ---- style: s = (y @ dense_w.T) * mul + b ; ssq = s^2 on ACT ----
            s_sb = small.tile([128, IC, BL], f32)
            ssq = small.tile([128, IC, BL], f32)
            for ct in range(IC):
                ps = pss.tile([128, BL], f32)
                for lc in range(4):
                    nc.tensor.matmul(
                        ps[:, :],
                        dwt_sb[:, lc, ct * 128:(ct + 1) * 128],
                        yt_sb[:, lc, :],
                        start=(lc == 0),
                        stop=(lc == 3),
                    )
                nc.scalar.activation(
                    s_sb[:, ct, :], ps[:, :], AF.Identity,
                    bias=db_sb[:, ct, :], scale=MUL_DENSE,
                )
                nc.scalar.activation(ssq[:, ct, :], s_sb[:, ct, :], AF.Square)

            # ---- x sample 0: fp32 quarter loads (many queues), scale+convert on ACT ----
            for qq in range(4):
                for ic in range(IC):
                    h0 = qq * 16
                    xr = xstage.tile([128, 16, W], f32, tag="xr")
                    nc.sync.dma_start(xr[:, :, :], x_d[0, ic * 128:(ic + 1) * 128, h0:h0 + 16, :])
                    nc.scalar.activation(
                        xs[:, 0, ic, h0:h0 + 16, :], xr[:, :, :], AF.Copy,
                        scale=s_sb[:, ic, 0:1],
                    )

            # ---- demod: T = wsq.T @ ssq ; D = 1/sqrt(T + eps') ----
            sqrt_t = small.tile([128, OC, BL], f32)
            d_sb = small.tile([128, OC, BL], f32)
            eps_sb = small.tile([128, 1], f32)
            nc.gpsimd.memset(eps_sb[:, :], EPS2)
            for ot in range(OC):
                ps = pss.tile([128, BL], f32)
                for ic in range(IC):
                    nc.tensor.matmul(
                        ps[:, :],
                        wsq[:, ic, ot * 128:(ot + 1) * 128],
                        ssq[:, ic, :],
                        start=(ic == 0),
                        stop=(ic == 3),
                    )
                nc.scalar.activation(
                    sqrt_t[:, ot, :], ps[:, :], AF.Sqrt,
                    bias=eps_sb[:, :], scale=1.0,
                )
                nc.vector.reciprocal(d_sb[:, ot, :], sqrt_t[:, ot, :])

            # ---- x sample 1: loads deferred behind first output stores so they
            #      don't steal DMA bandwidth from the critical prologue;
            #      scale+convert on DVE (free after wsq) ----
            def emit_x1():
                for qq in range(4):
                    for ic in range(IC):
                        h0 = qq * 16
                        xr = xstage.tile([128, 16, W], f32, tag="xr")
                        nc.sync.dma_start(xr[:, :, :], x_d[1, ic * 128:(ic + 1) * 128, h0:h0 + 16, :])
                        nc.vector.tensor_scalar_mul(
                            xs[:, 1, ic, h0:h0 + 16, :], xr[:, :, :], s_sb[:, ic, 1:2],
                        )

            # ---- main conv: 36 accumulating matmuls per (b, ot, row-tile) ----
            for b in range(BL):
                for ot in range(OC):
                    o0 = ot * 128
                    for r0, rows in zip(ROW_STARTS, ROW_COUNTS):
                        ps = psc.tile([128, 8, OW], f32, tag="convps")
                        first = True
                        for ic in range(IC):
                            for kh in range(KH):
                                for kw in range(KW):
                                    last = ic == IC - 1 and kh == KH - 1 and kw == KW - 1
                                    nc.tensor.matmul(
                                        ps[:, :rows, :],
                                        wbf[:, ic, kh * 3 + kw, o0:o0 + 128],
                                        xs[:, b, ic, r0 + kh:r0 + kh + rows, kw:kw + OW],
                                        start=first,
                                        stop=last,
                                    )
                                    first = False
                        ob = osb.tile([128, 8, OW], f32, tag="outsb")
                        nc.scalar.activation(
                            ob[:, :rows, :], ps[:, :rows, :], AF.Copy,
                            scale=d_sb[:, ot, b:b + 1],
                        )
                        nc.sync.dma_start(
                            out_d[b, o0:o0 + 128, r0:r0 + rows, :], ob[:, :rows, :]
                        )
                    if b == 0 and ot == 0:
                        emit_x1()

    nc.compile()
    return nc


def run(inputs, profile=False):
    """inputs: dict with full-size x/y/dense_w/dense_b/weight numpy arrays.
    Returns (out [16,512,62,62] float32, exec_time_ns or None)."""
    import ml_dtypes
    from concourse.bass_utils import run_bass_kernel_spmd

    if "nc" not in _cache:
        _cache["nc"] = build()
    nc = _cache["nc"]

    x = np.ascontiguousarray(np.asarray(inputs["x"], dtype=np.float32))
    y = np.ascontiguousarray(np.asarray(inputs["y"], dtype=np.float32))
    dense_w = np.asarray(inputs["dense_w"], dtype=np.float32)
    dense_b = np.asarray(inputs["dense_b"], dtype=np.float32)
    weight = np.asarray(inputs["weight"], dtype=np.float32)

    # weights pre-transposed to the [cin, tap, cout] layout the PE consumes,
    # in the bf16 the kernel computes with
    wtb = np.ascontiguousarray(
        weight.reshape(COUT, CIN, 9).transpose(1, 2, 0).astype(ml_dtypes.bfloat16))
    dwt = np.ascontiguousarray(dense_w.T)                          # [l, c]
    db = np.ascontiguousarray(dense_b.reshape(CIN, 1))

    in_maps = []
    for c in range(N_CORES):
        sl = slice(c * BL, (c + 1) * BL)
        in_maps.append({
            "x": x[sl],
            "wtb": wtb,
            "dwt": dwt,
            "yt": np.ascontiguousarray(y[sl].T),                   # [l, b]
            "db": db,
        })

    if profile:
        _ensure_ntff_hook()
    res = run_bass_kernel_spmd(
        nc, in_maps, core_ids=list(range(N_CORES)), trace=profile)
    out = np.concatenate([r["out"] for r in res.results], axis=0)
    return out, res.exec_time_ns


def kernel(**inputs) -> np.ndarray:
    out, _ = run(inputs)
    return out
